# revision 12
# baseline (speedup 1.0000x reference)
"""Trainium2 Bass kernel for nn_ModelNew_17411797418162.

Computation (per (b,s) sample):
  mixed = h_res @ x            # [4,4] @ [4,1024]
  out   = mixed * h_out[None,:] + h_post[:,None] * x

Sharding: pure data parallel over the leading batch dim B=8 -> 1 batch/core.

Per-core design (memory-bound, ~72MB HBM traffic -> ~201us roofline/core):
- Flatten (s, stream) -> rows: x_flat/out_flat [8192, 1024] f32; x loads and
  out stores are 2MB contiguous-row DMAs on the two HWDGE rings.
- The per-sample 4x4 GEMM runs on the PE as exact-fp32 block-diagonal
  [128,128] matmuls covering 32 samples each (K = 32 samples x 4 streams);
  the block-diagonal weights are scattered host-side (layout only) and kept
  resident in SBUF (one 4MB preload).
- h_out must be replicated x4 onto the stream rows (PSUM rows are
  (sample,stream), h_out is per-sample). Replicating via DRAM re-reads or
  SBUF->SBUF DMA costs ~25-50% extra traffic, so instead a constant
  replication matmul on the PE broadcasts it: h_out is decomposed host-side
  into 2 fp16 planes (hi, lo*2^10; subnormal-flushed entries zeroed) and
  multiplied by a constant [32->128] 0/1-pattern matrix (entries 1.0 resp.
  2^-10), accumulating in fp32 PSUM. Reconstruction rel err ~1e-7, and the
  fp16 planes halve h_out's HBM bytes.
- Epilogue: ScalarE evacuates the broadcast h_out PSUM, VectorE multiplies
  the mixed PSUM by it, ScalarE computes h_post*x via per-partition
  activation scale, VectorE adds. All engine time hides under the DMA
  stream (~250us/core measured steady-state vs PE ~120us, DVE ~150us,
  ACT ~130us warm).
Measured end-to-end (8 cores, steady state): ~235-255 us/pass,
max rel err ~1.6e-7 vs the fp32 reference.
"""
import numpy as np

import concourse.bacc as bacc
import concourse.tile as tile
import concourse.mybir as mybir
from concourse.bass_utils import run_bass_kernel_spmd

B, S, N, D = 8, 2048, 4, 1024
NCORES = 8
ROWS = S * N              # 8192 flattened rows per core
NSB = 16                  # super-blocks per core
SUBS = 4                  # sub-blocks (32 samples each) per super-block
F32 = mybir.dt.float32
BF16 = mybir.dt.bfloat16
FP16 = mybir.dt.float16

# h_out is decomposed host-side into 2 fp16 planes (hi, lo*2^10 - scaled to
# dodge fp16 subnormal flush) and replicated onto the 4 stream rows on-chip
# by a tiny constant matmul on the PE (entries 1.0 resp. 2^-10), accumulating
# the planes in fp32 PSUM. Reconstruction rel err ~1e-7.
HO_SPLITS = 2
LO_SCALE = 1024.0

_cache = {}


def build_program(iters: int = 1, mode: str = "full"):
    """Build the SPMD Bass program (one core's view). Cached per (iters, mode).

    mode: "full" = real kernel; ablations for bottleneck isolation:
      "dma"   = loads + store only (wrong output values)
      "novec" = loads + DVE/ACT epilogue, no matmuls (wrong values)
      "nope"  = full minus the E-broadcast matmuls (wrong values)
    """
    if (iters, mode) in _cache:
        return _cache[(iters, mode)]

    nc = bacc.Bacc("TRN2", target_bir_lowering=False, debug=False)
    x = nc.dram_tensor("x", [ROWS, D], F32, kind="ExternalInput")
    w = nc.dram_tensor("w", [64, 128, 128], F32, kind="ExternalInput")
    hoall = nc.dram_tensor("hoall", [S, HO_SPLITS, D], FP16,
                           kind="ExternalInput")
    e4 = nc.dram_tensor("e4", [128, HO_SPLITS * 128], FP16,
                        kind="ExternalInput")
    hp = nc.dram_tensor("hp", [128, 64], F32, kind="ExternalInput")
    out = nc.dram_tensor("out", [ROWS, D], F32, kind="ExternalOutput")

    with tile.TileContext(nc) as tc:
        with (
            tc.tile_pool(name="const", bufs=1) as cpool,
            tc.tile_pool(name="big", bufs=3) as bpool,
            tc.tile_pool(name="mid", bufs=3) as mpool,
            tc.tile_pool(name="psum", bufs=4, space="PSUM") as ppool,
        ):
            hp_all = cpool.tile([128, 64], F32)
            nc.gpsimd.dma_start(hp_all[:], hp.ap())
            e4_t = cpool.tile([128, HO_SPLITS * 128], FP16)
            nc.gpsimd.dma_start(e4_t[:], e4.ap())
            # resident block-diag weights: w_all[r, (b, c)] = w[b, r, c]
            w_all = cpool.tile([128, 64 * 128], F32)
            nc.gpsimd.dma_start(
                w_all[:].rearrange("r (b c) -> r b c", b=64),
                w.ap().rearrange("b r c -> r b c"))

            def body():
                for sb in range(NSB):
                    # x rows 512*sb .. 512*(sb+1), tiled [p=128, (k=4, d=1024)]
                    x_t = bpool.tile([128, SUBS * D], F32, tag="x")
                    src = x.ap()[512 * sb:512 * (sb + 1), :].rearrange(
                        "(k p) d -> p k d", k=SUBS)
                    nc.sync.dma_start(
                        x_t[:].rearrange("p (k d) -> p k d", k=SUBS), src)

                    # h_out bf16 planes for these 128 samples, one DMA:
                    # h_all_t[p, (s, d)] = hoall[128*sb + p, s, d]
                    h_all_t = mpool.tile([128, HO_SPLITS * D], FP16, tag="hoal")
                    nc.gpsimd.dma_start(
                        h_all_t[:].rearrange("p (s d) -> p s d", s=HO_SPLITS),
                        hoall.ap()[128 * sb:128 * (sb + 1)])
                    ho_ts = [h_all_t[:, D * s:D * (s + 1)]
                             for s in range(HO_SPLITS)]

                    out_sb = bpool.tile([128, SUBS * D], F32, tag="out_sb")

                    for k in range(SUBS):
                        if mode == "dma":
                            continue
                        ho4_t = mpool.tile([128, D], F32, tag="ho4")
                        if mode == "full":
                            # broadcast h_out onto stream rows via 0/1 matmul
                            psh = ppool.tile([128, D], F32, tag="ps")
                            for c in range(2):
                                for s in range(HO_SPLITS):
                                    lhsE = e4_t[32 * k:32 * (k + 1),
                                                128 * s:128 * (s + 1)]
                                    nc.tensor.matmul(
                                        psh[:, 512 * c:512 * (c + 1)],
                                        lhsE,
                                        ho_ts[s][32 * k:32 * (k + 1),
                                                 512 * c:512 * (c + 1)],
                                        start=(s == 0),
                                        stop=(s == HO_SPLITS - 1),
                                        tile_position=(32 * k, 0))
                            nc.scalar.copy(ho4_t[:], psh[:])
                        else:
                            nc.scalar.copy(
                                ho4_t[:].bitcast(FP16)[:, 0:D], ho_ts[0])

                        t_t = mpool.tile([128, D], F32, tag="t")
                        if mode in ("full", "nope"):
                            ps = ppool.tile([128, D], F32, tag="ps")
                            blk = SUBS * sb + k
                            lhsT = w_all[:, 128 * blk:128 * (blk + 1)]
                            for c in range(2):
                                nc.tensor.matmul(
                                    ps[:, 512 * c:512 * (c + 1)],
                                    lhsT,
                                    x_t[:, D * k + 512 * c:
                                        D * k + 512 * (c + 1)],
                                    start=True, stop=True)
                            nc.vector.tensor_mul(t_t[:], ps[:], ho4_t[:])
                        else:
                            nc.vector.tensor_mul(
                                t_t[:], x_t[:, D * k:D * (k + 1)], ho4_t[:])

                        p_t = mpool.tile([128, D], F32, tag="p")
                        col = SUBS * sb + k
                        nc.scalar.activation(
                            p_t[:], x_t[:, D * k:D * (k + 1)],
                            mybir.ActivationFunctionType.Copy,
                            scale=hp_all[:, col:col + 1])

                        nc.vector.tensor_add(
                            out_sb[:, D * k:D * (k + 1)], t_t[:], p_t[:])

                    if mode == "dma":
                        src_sb = x_t
                    else:
                        src_sb = out_sb
                    dst = out.ap()[512 * sb:512 * (sb + 1), :].rearrange(
                        "(k p) d -> p k d", k=SUBS)
                    nc.scalar.dma_start(
                        dst, src_sb[:].rearrange("p (k d) -> p k d", k=SUBS))

            if iters == 1:
                body()
            else:
                with tc.For_i(0, iters, 1):
                    body()

    nc.compile()
    _cache[(iters, mode)] = nc
    return nc


def make_in_maps(x, h_res, h_out, h_post):
    """Split full inputs into per-core input maps (host-side, layout only)."""
    x = np.ascontiguousarray(x, dtype=np.float32)
    h_res = np.ascontiguousarray(h_res, dtype=np.float32)
    h_out = np.ascontiguousarray(h_out, dtype=np.float32)
    h_post = np.ascontiguousarray(h_post, dtype=np.float32)

    # stream-replication matrices: e4[32k+p, 128s + 4p+i] = plane s weight
    e4 = np.zeros((128, HO_SPLITS * 128), np.float16)
    q = np.arange(128)
    wts = [1.0, 1.0 / LO_SCALE]
    for s in range(HO_SPLITS):
        for i in range(4):
            e4[q, 128 * s + 4 * (q % 32) + i] = wts[s]

    in_maps = []
    for c in range(NCORES):
        xc = x[c].reshape(ROWS, D)
        # Block-diagonal weights: W[b, 4p+j, 4p+i] = h_res[c, 32b+p, i, j]
        hr = h_res[c].reshape(64, 32, 4, 4)          # [b, p, i, j]
        Wb = np.zeros((64, 32, 4, 32, 4), np.float32)  # [b, (p,j), (p,i)]
        idx = np.arange(32)
        # advanced indexing: result axes (idx-bcast, b, j, i)
        Wb[:, idx, :, idx, :] = hr.transpose(1, 0, 3, 2)
        Wc = Wb.reshape(64, 128, 128)
        hpc = np.ascontiguousarray(
            h_post[c].reshape(64, 128).T)             # hp[p, b] = flat[128b+p]
        m = {"x": xc, "w": Wc, "hp": hpc, "e4": e4}
        planes = np.empty((S, HO_SPLITS, D), np.float16)
        hoc = h_out[c]
        hi = hoc.astype(np.float16)
        hi[np.abs(hi.astype(np.float32)) < 2.0 ** -14] = 0
        lo = ((hoc - hi.astype(np.float32)) * LO_SCALE).astype(np.float16)
        lo[np.abs(lo.astype(np.float32)) < 2.0 ** -14] = 0
        planes[:, 0, :] = hi
        planes[:, 1, :] = lo
        m["hoall"] = planes
        in_maps.append(m)
    return in_maps


def kernel(x, h_res, h_out, h_post):
    nc = build_program(iters=1)
    in_maps = make_in_maps(x, h_res, h_out, h_post)
    res = run_bass_kernel_spmd(nc, in_maps, list(range(NCORES)))
    out = np.stack([res.results[c]["out"].reshape(S, N, D)
                    for c in range(NCORES)])
    return out.astype(np.float32)
